# revision 50
# baseline (speedup 1.0000x reference)
"""Mie scattering phase function on 8 Trainium2 NeuronCores.

Math: the reference's S1/S2 amplitudes are polynomials in mu of degree <= NMAX+1.
We parity-split S(mu) = E(mu^2) + mu*O(mu^2) and fit the 8 scalar targets
(E/O parts of Re/Im S1/S2, pre-scaled by 1/sqrt(2 x^2)) in a two-level Chebyshev
product basis on uh = 2 mu^2 - 1:  { T_j(T_8(uh)) * T_r(uh) : j,r in 0..7 }
(64 features, spans degree <= 63).

Device kernel per core (131072 angles, 2 megatiles of 128x512 angle columns):
  - Base Chebyshev bands via binary doubling (T_2m = 2 T_m^2 - 1 with the
    square on ACT, T_{m+n} = 2 T_m T_n - T_{m-n} with mul on GPSIMD and a
    fused affine-subtract on DVE): ~14-deep critical path across 3 engines
    instead of a 31-op serial DVE chain (chained ops cost ~1us latency each).
  - Features: one 8-band product op per j level (Tv_j broadcast along a
    stride-0 inner dim), fp16, k-inner layout F[p, c*64 + k] so each column
    pair is a contiguous [128, 128] transpose input.
  - PE transpose (identity matmul) of [128 angles, 2cols x 64 bands] -> PSUM
    (4 transposes per 2KB bank), batched copyback of 8 units to SBUF
    (alternating DVE/ACT), then ONE matmul per column pair: lhsT = F^T
    [128=(2x64 bands), 128 angles], rhs = packed C [128, 16] (block-diagonal:
    rows 0-63 -> outputs 0-7 for even col, rows 64-127 -> outputs 8-15 for
    odd col). fp16 throughout; PSUM accumulates fp32.
  - Batched epilogue per 2-bank drain group (5 ops): S_t = E_t + mu*O_t for
    all t at once, square on ACT, pairwise adds -> phase = sum_t S_t^2.
No xbar DMA transposes (the old bottleneck: ~1.4us per 32KB tile on HW) and
no all-engine barriers in the reps=1 graph.
"""
import math
from contextlib import ExitStack

import numpy as np

NMAX = 135
R = 8
J = 8
NFEAT = R * J  # 64
N_ANGLES = 1048576
N_CORES = 8
PER_CORE = N_ANGLES // N_CORES  # 131072
P = 128
MEGA_COLS = 256                   # angle columns per megatile
N_MEGA = PER_CORE // (P * MEGA_COLS)  # 4
NUNIT = MEGA_COLS // 2            # column pairs per megatile
UNITS_PER_DRAIN = 64              # pairs per PSUM drain group (2 banks)
N_DRAIN = NUNIT // UNITS_PER_DRAIN  # 2


# ----------------------------------------------------------------------------
# Host-side math (float64): replicate the reference recurrences + basis fit
# ----------------------------------------------------------------------------

def _riccati_f64(z):
    z = complex(z)
    inv = 1.0 / (z + 1e-12)
    psi = np.zeros(NMAX + 2, np.complex128)
    chi = np.zeros(NMAX + 2, np.complex128)
    psi[0] = np.sin(z)
    psi[1] = psi[0] * inv - np.cos(z)
    chi[0] = -np.cos(z)
    chi[1] = np.cos(z) * inv - np.sin(z)
    for n in range(2, NMAX + 2):
        coef = (2.0 * n - 1.0) * inv
        psi[n] = coef * psi[n - 1] - psi[n - 2]
        chi[n] = coef * chi[n - 1] - chi[n - 2]
    xi = psi - 1j * chi
    i = np.arange(1, NMAX + 1, dtype=np.float64)
    psi_prime = np.concatenate([[np.cos(z)], psi[:NMAX] - i * inv * psi[1:NMAX + 1]])
    xi_prime = np.concatenate(
        [[np.cos(z) + 1j * np.sin(z)], xi[:NMAX] - i * inv * xi[1:NMAX + 1]])
    return psi, xi, psi_prime, xi_prime


def _mie_ab_f64(x, m):
    psi, xi, psip, xip = _riccati_f64(x)
    psm, _, psmp, _ = _riccati_f64(m * x)
    s = slice(1, NMAX + 1)
    an = (m * psm[s] * psip[s] - psi[s] * psmp[s]) / \
         (m * psm[s] * xip[s] - xi[s] * psmp[s] + 1e-30)
    bn = (psm[s] * psip[s] - m * psi[s] * psmp[s]) / \
         (psm[s] * xip[s] - m * xi[s] * psmp[s] + 1e-30)
    return an, bn


def _s1s2_f64(mu, x, m):
    an, bn = _mie_ab_f64(x, m)
    n = np.arange(1, NMAX + 1, dtype=np.float64)
    f = (2.0 * n + 1.0) / (n * (n + 1.0))
    fa, fb = f * an, f * bn
    mu = np.asarray(mu, np.float64)
    pi_all = np.zeros((NMAX, mu.size))
    tau_all = np.zeros((NMAX, mu.size))
    pi_all[0] = 3.0 * mu
    tau_all[0] = mu
    p1, p2 = 3.0 * mu, np.ones_like(mu)
    for k in range(2, NMAX + 1):
        nn = float(k)
        p = ((2 * nn + 1) * mu * p1 - (nn + 1) * p2) / nn
        t = nn * mu * p - (nn + 1) * p1
        pi_all[k - 1] = p
        tau_all[k - 1] = t
        p1, p2 = p, p1
    S1 = fa @ pi_all + fb @ tau_all
    S2 = fa @ tau_all + fb @ pi_all
    return S1, S2


def _cheb_T(k, x):
    return np.cos(k * np.arccos(np.clip(x, -1.0, 1.0)))


def _fit_coeffs(wavelength, radius, m_real, m_imag):
    """Returns C (NFEAT, 8) float64 — columns: E/O of S1r,S1i,S2r,S2i scaled."""
    x = 2.0 * math.pi * radius / (wavelength * 1e-9)
    m = m_real + 1j * m_imag
    M = 1024
    uh = np.cos((np.arange(M) + 0.5) * np.pi / M)
    u = (uh + 1.0) / 2.0
    mu = np.sqrt(u)
    S1p, S2p = _s1s2_f64(mu, x, m)
    S1m, S2m = _s1s2_f64(-mu, x, m)
    scale = 1.0 / math.sqrt(2.0 * x * x)
    targets = []
    pairs = ((S1p.real, S1m.real), (S1p.imag, S1m.imag),
             (S2p.real, S2m.real), (S2p.imag, S2m.imag))
    for Sp, Sm in pairs:                       # E parts: columns 0..3
        targets.append((Sp + Sm) / 2.0 * scale)
    for Sp, Sm in pairs:                       # O parts: columns 4..7
        targets.append((Sp - Sm) / (2.0 * mu + 1e-300) * scale)
    T = np.stack([_cheb_T(r, uh) for r in range(R + 1)])
    Tv = np.stack([_cheb_T(j, T[R]) for j in range(J)])
    B = np.zeros((M, NFEAT))
    for j in range(J):
        for r in range(R):
            B[:, j * R + r] = Tv[j] * T[r]
    C, *_ = np.linalg.lstsq(B, np.stack(targets, axis=1), rcond=None)
    return C


# ----------------------------------------------------------------------------
# Device kernel (Bass / Tile)
# ----------------------------------------------------------------------------

_CACHE = {}


def _build_nc(reps=1, skip=(), xbar_mod=0):
    skip = set(skip)
    import concourse.bass as bass
    import concourse.mybir as mybir
    import concourse.tile as tile
    from concourse import bacc, masks

    f32 = mybir.dt.float32
    f16 = mybir.dt.float16
    AOP = mybir.AluOpType

    nc = bacc.Bacc("TRN2", target_bir_lowering=False, debug=False)
    mu_d = nc.dram_tensor("mu", [PER_CORE], f32, kind="ExternalInput").ap()
    cpk_d = nc.dram_tensor("cpk", [P, 16], f16, kind="ExternalInput").ap()
    ph_d = nc.dram_tensor("phase", [PER_CORE], f32, kind="ExternalOutput").ap()

    mu_v = mu_d.rearrange("(m p c) -> m p c", p=P, c=MEGA_COLS)
    ph_v = ph_d.rearrange("(m p c) -> m p c", p=P, c=MEGA_COLS)

    with tile.TileContext(nc) as tc, ExitStack() as ctx:
        const_p = ctx.enter_context(tc.tile_pool(name="const", bufs=1))
        fstore_p = ctx.enter_context(tc.tile_pool(name="fstore", bufs=2))
        tbase_p = ctx.enter_context(tc.tile_pool(name="tbase", bufs=2))
        mu_p = ctx.enter_context(tc.tile_pool(name="mu", bufs=2))
        ph_p = ctx.enter_context(tc.tile_pool(name="ph", bufs=2))
        tmp_p = ctx.enter_context(tc.tile_pool(name="tmp", bufs=1))
        sq_p = ctx.enter_context(tc.tile_pool(name="sq", bufs=2))
        ftT_p = ctx.enter_context(tc.tile_pool(name="ftT", bufs=2))
        psum_mm = ctx.enter_context(tc.tile_pool(name="psmm", bufs=2, space="PSUM"))
        psum_tr = ctx.enter_context(tc.tile_pool(name="pstr", bufs=2, space="PSUM"))

        cpk_sb = const_p.tile([P, 16], f16)
        nc.sync.dma_start(cpk_sb[:], cpk_d)
        ident = const_p.tile([P, P], f16)
        masks.make_identity(nc, ident[:])

        # guards: last epilogue instruction that read each psum_mm buf
        guard = [None, None]

        rep_cm = tc.For_i(0, reps, 1) if reps > 1 else None
        if rep_cm is not None:
            rep_cm.__enter__()
            if "bar" not in skip:
                # one barrier per rep: collapses cross-rep wait fan-in
                # (F bufs=2 covers both megatiles within a rep)
                tc.strict_bb_all_engine_barrier()
        def make_stage_a(mt):
            """Allocate tiles and build the list of op-emitting thunks for
            megatile mt's input DMA + Chebyshev bases + feature products.
            Thunks are interleaved into the previous megatile's PE pipeline
            so the lead-in latency hides under matmul work."""
            ops = []
            mu_t = mu_p.tile([P, MEGA_COLS], f32)
            tb8 = tbase_p.tile([P, 8 * MEGA_COLS], f32, tag="tb8")
            tb8v = tb8[:].rearrange("p (c r) -> p c r", r=8)
            t8 = tbase_p.tile([P, MEGA_COLS], f32, tag="t8")
            tvb = tbase_p.tile([P, 6 * MEGA_COLS], f32, tag="tvb")
            tvb3 = tvb[:].rearrange("p (j c) -> p j c", c=MEGA_COLS)
            u_t = tmp_p.tile([P, MEGA_COLS], f32, tag="utile")
            F = fstore_p.tile([P, NFEAT * MEGA_COLS], f16)
            st = {"mu": mu_t, "F": F}

            ops.append(lambda: nc.sync.dma_start(mu_t[:], mu_v[mt]))
            if "rec" in skip:
                return ops, st
            # Binary-doubling Chebyshev: T_2m = 2 T_m^2 - 1 (square on ACT,
            # affine on DVE); T_{m+n} = 2 T_m T_n - T_{m-n} (mul on GPSIMD,
            # fused affine-sub on DVE). Critical path ~14 ops across 3
            # engines (vs 31 serial DVE ops for the linear recurrence).
            ops.append(lambda: nc.vector.memset(tb8v[:, :, 0], 1.0))  # T_0
            ops.append(lambda: nc.vector.tensor_mul(u_t[:], mu_t[:], mu_t[:]))
            T = {}

            def dbl(m, dst):
                sq = tmp_p.tile([P, MEGA_COLS], f32, tag=f"sq{(2*m) % 3}")
                ops.append(lambda: nc.scalar.square(sq[:], T[m]))
                ops.append(lambda: nc.vector.tensor_scalar(
                    dst, sq[:], 2.0, -1.0, AOP.mult, AOP.add))
                T[2 * m] = dst

            def add_(m, n, dst):
                pr = tmp_p.tile([P, MEGA_COLS], f32, tag=f"pr{(m+n) % 3}")
                ops.append(lambda: nc.gpsimd.tensor_mul(pr[:], T[m], T[n]))
                ops.append(lambda: nc.vector.scalar_tensor_tensor(
                    dst, pr[:], 2.0, T[m - n], AOP.mult, AOP.subtract))
                T[m + n] = dst

            ops.append(lambda: nc.vector.tensor_scalar(
                tb8v[:, :, 1], u_t[:], 2.0, -1.0, AOP.mult, AOP.add))
            T[1] = tb8v[:, :, 1]
            dbl(1, tb8v[:, :, 2])
            add_(2, 1, tb8v[:, :, 3])
            dbl(2, tb8v[:, :, 4])
            add_(3, 2, tb8v[:, :, 5])
            dbl(3, tb8v[:, :, 6])
            add_(4, 3, tb8v[:, :, 7])
            dbl(4, t8[:])
            dbl(8, tvb3[:, 0])          # Tv_2 = T_16
            add_(16, 8, tvb3[:, 1])     # Tv_3 = T_24
            dbl(16, tvb3[:, 2])         # Tv_4 = T_32
            add_(24, 16, tvb3[:, 3])    # Tv_5 = T_40
            dbl(24, tvb3[:, 4])         # Tv_6 = T_48
            add_(32, 24, tvb3[:, 5])    # Tv_7 = T_56

            # features: fp16, k-inner store F[p, c*64 + j*8 + r]
            # one op per j: F5[:, j] = Tv_j (bcast over r) * T_{0..7}
            F5 = F[:].rearrange("p (c j r) -> p j c r", j=J, r=8)
            if "feat" not in skip:
                ops.append(lambda: nc.scalar.copy(F5[:, 0], tb8v[:]))
                for j in range(1, J):
                    tv_ap = t8[:] if j == 1 else tvb3[:, j - 2]
                    tv_b = tv_ap.rearrange("p (c one) -> p c one",
                                           one=1).broadcast_to(
                        [P, MEGA_COLS, 8])
                    eng = nc.gpsimd if j <= 4 else nc.vector
                    ops.append(lambda eng=eng, tv_b=tv_b, j=j:
                               eng.tensor_mul(F5[:, j], tv_b, tb8v[:]))
            return ops, st

        def stage_b(mt, st, a_feed):
            """Transpose/matmul/drain pipeline for megatile mt, interleaving
            next-megatile stage-A thunks after each 8-unit group."""
            nonlocal ftT_static
            mu_t, F = st["mu"], st["F"]
            # transpose input: unit q = cols (2q, 2q+1), contiguous 128 bands
            F4 = F[:].rearrange("p (q m) -> p q m", m=2 * NFEAT)
            ps = None
            if "tr" in skip and ftT_static is None:
                ftT_static = ftT_p.tile([P, 8 * P], f16, tag="static")
                nc.vector.memset(ftT_static[:], 0.25)
            for q in range(NUNIT):
                g, u = q // UNITS_PER_DRAIN, q % UNITS_PER_DRAIN
                s = q % 8
                if u == 0:
                    ps = psum_mm.tile([P, UNITS_PER_DRAIN * 16], f32)
                    ps4 = ps[:].rearrange("p (u b o) -> p u b o", b=2, o=8)
                grp = q // 8
                use_xbar = xbar_mod > 0 and (grp % xbar_mod == xbar_mod - 1)
                if "tr" not in skip and use_xbar:
                    # xbar DMA transpose straight to SBUF (2 hwdge queues),
                    # bypassing PSUM + copyback; offloads the PE
                    if s == 0:
                        ftT8 = ftT_p.tile([P, 8 * P], f16)
                    eng = nc.sync if (q % 2 == 0) else nc.scalar
                    eng.dma_start_transpose(ftT8[:, s * P:(s + 1) * P],
                                            F4[:, q])
                elif "tr" not in skip:
                    # 8 transposes share 2 PSUM banks; single batched copyback
                    if s == 0:
                        pst8 = psum_tr.tile([P, 8 * P], f16)
                        pst8v = pst8[:].rearrange("p (s a) -> p s a", s=8)
                    tr = nc.tensor.matmul(pst8v[:, s], F4[:, q], ident[:],
                                          is_transpose=True,
                                          start=(s % 4 == 0), stop=(s % 4 == 3))
                    if s % 4 != 0:
                        tile.add_dep_helper(tr.ins, prev_tr.ins, sync=False,
                                            reason="transpose bank order")
                    prev_tr = tr
                    if s == 7:
                        ftT8 = ftT_p.tile([P, 8 * P], f16)
                        if (q // 8) % 2 == 0:
                            nc.vector.tensor_copy(ftT8[:], pst8[:])
                        else:
                            nc.scalar.copy(ftT8[:], pst8[:])
                elif s == 7:
                    ftT8 = ftT_static
                if "mm" in skip:
                    continue
                if s == 7:
                    # issue the 8 matmuls for units q-7..q
                    for si in range(8):
                        qq = q - 7 + si
                        uu = qq % UNITS_PER_DRAIN
                        start = (uu % 32 == 0)
                        mm = nc.tensor.matmul(
                            ps4[:, uu], ftT8[:, si * P:(si + 1) * P], cpk_sb[:],
                            start=start, stop=(uu % 32 == 31))
                        if start and guard[g % 2] is not None:
                            tile.add_dep_helper(mm.ins, guard[g % 2].ins,
                                                sync=True,
                                                reason="bank reuse after epi")
                        if not start:
                            tile.add_dep_helper(mm.ins, prev_mm.ins, sync=False,
                                                reason="psum bank order")
                        prev_mm = mm
                    # interleave a few next-megatile stage-A ops so their
                    # chain latency hides under this megatile's PE pipeline
                    for _ in range(3):
                        a_op = next(a_feed, None)
                        if a_op is not None:
                            a_op()

                if u == UNITS_PER_DRAIN - 1:
                    # --- drain epilogue for group g: cols 128g..128g+127 ---
                    # S_t = E_t + mu*O_t (batched over t); phase = 4*avg(S_t^2)
                    # (the 2x S-scale is folded into C host-side)
                    cs = slice(P * g, P * (g + 1))
                    mu4 = mu_t[:, cs].rearrange(
                        "p (u b one) -> p u b one", b=2, one=1).broadcast_to(
                        [P, UNITS_PER_DRAIN, 2, 4])
                    s_t = sq_p.tile([P, UNITS_PER_DRAIN * 8], f32, tag="stile")
                    s4 = s_t[:].rearrange("p (u b t) -> p u b t", b=2, t=4)
                    sq_t = sq_p.tile([P, UNITS_PER_DRAIN * 8], f32, tag="sqtile")
                    if g == 0:
                        ph_t = ph_p.tile([P, MEGA_COLS], f32)
                    nc.vector.tensor_mul(s4, mu4, ps4[:, :, :, 4:8])
                    guard[g % 2] = nc.vector.tensor_add(s4, s4, ps4[:, :, :, 0:4])
                    nc.scalar.square(sq_t[:], s_t[:])
                    sq4 = sq_t[:].rearrange("p (c t) -> p c t", t=4)
                    pr_t = sq_p.tile([P, UNITS_PER_DRAIN * 4], f32, tag="prtile")
                    pr3 = pr_t[:].rearrange("p (c t) -> p c t", t=2)
                    nc.vector.tensor_add(pr3, sq4[:, :, 0:2], sq4[:, :, 2:4])
                    nc.vector.tensor_add(ph_t[:, cs], pr3[:, :, 0], pr3[:, :, 1])
                    if g == N_DRAIN - 1:
                        nc.sync.dma_start(ph_v[mt], ph_t[:])

        ftT_static = None
        a_ops, a_st = make_stage_a(0)
        for a_op in a_ops:
            a_op()
        for mt in range(N_MEGA):
            if mt + 1 < N_MEGA:
                n_ops, n_st = make_stage_a(mt + 1)
            else:
                n_ops, n_st = [], None
            feed = iter(n_ops)
            stage_b(mt, a_st, feed)
            for a_op in feed:
                a_op()
            a_st = n_st

        if rep_cm is not None:
            rep_cm.__exit__(None, None, None)

    nc.compile()
    return nc


def _get_compiled():
    if "nc" not in _CACHE:
        _CACHE["nc"] = _build_nc()
    return _CACHE["nc"]


def _make_in_maps(mu, wavelength, radius, m_real, m_imag):
    C = _fit_coeffs(wavelength, radius, m_real, m_imag)
    cpk = np.zeros((P, 16), np.float16)
    cpk[0:NFEAT, 0:8] = C.astype(np.float16)
    cpk[NFEAT:2 * NFEAT, 8:16] = C.astype(np.float16)
    shards = mu.reshape(N_CORES, PER_CORE)
    return [{"mu": shards[i], "cpk": cpk} for i in range(N_CORES)]


def kernel(cos_theta, wavelength, radius, m_real, m_imag):
    from concourse.bass_utils import run_bass_kernel_spmd

    mu = np.asarray(cos_theta, np.float32).reshape(-1)
    assert mu.size == N_ANGLES
    in_maps = _make_in_maps(mu, float(np.asarray(wavelength)),
                            float(np.asarray(radius)),
                            float(np.asarray(m_real)),
                            float(np.asarray(m_imag)))
    nc = _get_compiled()
    import os
    trace = bool(os.environ.get("MIE_TRACE"))
    res = run_bass_kernel_spmd(nc, in_maps, list(range(N_CORES)), trace=trace)
    _CACHE["last_res"] = res
    out = np.concatenate([np.asarray(res.results[i]["phase"], np.float32)
                          for i in range(N_CORES)])
    return out
